# revision 18
# baseline (speedup 1.0000x reference)
"""ALiBi causal attention (B=2, T=2048, C=1024, H=16, D=64, fp32) on 8 trn2 cores.

Sharding: core i -> batch b = i//4, head-group g = i%4 (4 heads = 256 channels).
Each core computes Q/K/V projections for its head slice, causal ALiBi attention,
and a partial output projection; host sums the 4 partials per batch.

Device dataflow (per core), all matmuls in fp32r (fp22 single-pass):
  phase 1: xT (C,T) in SBUF; QT/KT in (d,t) layout with aug rows:
           QTe rows: 64 = -slope*t/scale, 65 = 1, 66 = 1
           KTe rows: 64 = 1, 65 = hi(slope*s/scale), 66 = lo remainder
           (hi/lo split keeps the ALiBi s-term exact through fp22 truncation;
           the t-term's fp22 error is constant per t and cancels in softmax).
           V in (t,d) layout with an appended ones column (denominator trick).
  phase 2: per (head, 512-wide q-block qb, pair of 128-wide s-tiles):
           ST pair -> one 1024-wide ACT exp (bias-free) -> PT
           causal mask on diagonal tiles via gpsimd affine_select (fill 0);
           diagonal tiles width-trimmed to >=256 cols.
           AV += V_aug^T @ PT -> (65, t): rows 0..63 numerator, row 64 denom
           normalize: recip(den) -> PE ones-broadcast -> multiply
  phase 3: out_partial = attn_outT^T @ WoT -> DMA to DRAM

One unified PSUM pool (8 banks exactly: p1 2 + st 2x2 + av 1 + bc 1) and
scoped SBUF pools keep the three phases free to overlap on the engines.
"""

import sys

import numpy as np

for _p in ("/opt/trn_rl_repo", "/root/.axon_site/_ro/trn_rl_repo"):
    try:
        import concourse  # noqa: F401
        break
    except ImportError:
        if _p not in sys.path:
            sys.path.insert(0, _p)

B, T, C, H, D = 2, 2048, 1024, 16, 64
HPC = 4          # heads per core
CS = HPC * D     # 256 channels per core
SCALE = D ** -0.5
NCORES = 8
KAUG = 67        # 64 head dims + t-term row + hi/lo s-term rows


def _slopes(n_heads: int) -> np.ndarray:
    i = np.arange(1, n_heads + 1, dtype=np.float64)
    return np.power(2.0, -8.0 * i / n_heads).astype(np.float32)


def _trunc22(v: np.ndarray) -> np.ndarray:
    """Truncate fp32 to 11 mantissa bits so the value is exactly representable
    in the PE's internal FP22 (e10m11) format -- the hi part of the hi/lo
    ALiBi split must survive the fp32r read unchanged."""
    u = np.asarray(v, np.float32).view(np.uint32)
    return (u & np.uint32(0xFFFFF000)).view(np.float32)


_PROGRAM = None

# diagonal-tile trimming: r = st - 4*qb, computed column window [off, 512)
_DIAG_OFF = [0, 128, 256, 256]

# Interleaved head sharding: core g takes heads {g, g+4, g+8, g+12} so every
# core sees the same slope spread. For head slot j the smallest slope across
# cores is 2^-(2j+2); s-tiles with sigma*gap > 40 contribute < e^-35 of the
# softmax mass and are skipped. K = max trailing s-tiles kept per 512 block.
_SKIP_K = [2, 5, 16, 16]  # ceil(40 / (2^-(2j+2) * 128)), capped at 16


def _st_start(j_slot: int, qb: int) -> int:
    return max(0, 4 * qb - _SKIP_K[j_slot])


def _build_program():
    """Build the single-core Bass program (same program on all 8 cores)."""
    from contextlib import ExitStack

    import concourse.tile as tile
    from concourse import bacc, mybir

    f32 = mybir.dt.float32
    f32r = mybir.dt.float32r
    EXP = mybir.ActivationFunctionType.Exp

    nc = bacc.Bacc("TRN2", target_bir_lowering=False, debug=False,
                   num_devices=NCORES)
    xT = nc.declare_dram_parameter("xT", [C, T], f32, isOutput=False)
    wqT = nc.declare_dram_parameter("wqT", [C, CS], f32, isOutput=False)
    wkT = nc.declare_dram_parameter("wkT", [C, CS], f32, isOutput=False)
    wvT = nc.declare_dram_parameter("wvT", [C, CS], f32, isOutput=False)
    woT = nc.declare_dram_parameter("woT", [CS, C], f32, isOutput=False)
    qaug = nc.declare_dram_parameter("qaug", [3 * HPC, T], f32, isOutput=False)
    kaug = nc.declare_dram_parameter("kaug", [3 * HPC, T], f32, isOutput=False)
    out = nc.declare_dram_parameter("out", [T, C], f32, isOutput=True)

    KT_C = C // 128   # 8 contraction tiles for projections
    NQT = T // 128    # 16 q/s tiles
    NQB = T // 512    # 4 q blocks

    with nc.allow_low_precision(reason="f32r is 4-byte; rounding only at PE"), \
         tile.TileContext(nc) as tc, ExitStack() as ctx:
        # ---- long-lived SBUF + the single PSUM pool ----
        qk_pool = ctx.enter_context(tc.tile_pool(name="qk", bufs=1))
        psum = ctx.enter_context(tc.tile_pool(name="psum", bufs=1, space="PSUM"))
        qt_t = [qk_pool.tile([KAUG, T], f32r, tag=f"qt{h}", name=f"qt{h}")
                for h in range(HPC)]
        kt_t = [qk_pool.tile([KAUG, T], f32r, tag=f"kt{h}", name=f"kt{h}")
                for h in range(HPC)]
        v_t = qk_pool.tile([128, NQT, HPC, 65], f32r)      # V + ones column
        attn_t = [qk_pool.tile([128, T], f32r, tag=f"at{i}", name=f"at{i}")
                  for i in range(2)]
        ones_sb = qk_pool.tile([1, 64], f32r)

        nc.vector.memset(ones_sb[:].bitcast(f32), 1.0)
        nc.vector.memset(v_t[:].bitcast(f32), 1.0)  # ones col preset
        for h in range(HPC):
            nc.gpsimd.dma_start(out=qt_t[h][64:67, :],
                                in_=qaug[3 * h:3 * h + 3, :].bitcast(f32r))
            nc.gpsimd.dma_start(out=kt_t[h][64:67, :],
                                in_=kaug[3 * h:3 * h + 3, :].bitcast(f32r))

        # ---- phase 1: projections (xt/w scratch dies at end of phase) ----
        with tc.tile_pool(name="xt", bufs=1) as xt_pool:
            xt_sb = xt_pool.tile([128, KT_C, T], f32r)
            wq_sb = xt_pool.tile([128, KT_C, CS], f32r)
            wk_sb = xt_pool.tile([128, KT_C, CS], f32r)
            wv_sb = xt_pool.tile([128, KT_C, CS], f32r)
            for (w_sb, src) in ((wv_sb, wvT), (wq_sb, wqT), (wk_sb, wkT)):
                nc.sync.dma_start(
                    out=w_sb[:],
                    in_=src.rearrange("(k p) c -> p k c", p=128).bitcast(f32r))
            xt_view = xT.rearrange("(k p) t -> p k t", p=128).bitcast(f32r)
            for tch in range(NQB):
                nc.sync.dma_start(out=xt_sb[:, :, tch * 512:(tch + 1) * 512],
                                  in_=xt_view[:, :, tch * 512:(tch + 1) * 512])

            # V first so attention can start early: (t on partitions, d free)
            for st in range(NQT):
                ps = psum.tile([128, 512], f32, tag="p1", bufs=2, name=f"vps{st}")
                for k in range(KT_C):
                    nc.tensor.matmul(
                        ps[:, 0:CS],
                        lhsT=xt_sb[:, k, st * 128:(st + 1) * 128],
                        rhs=wv_sb[:, k, :],
                        start=(k == 0), stop=(k == KT_C - 1),
                    )
                nc.scalar.copy(
                    v_t[:, st, :, 0:64],
                    ps[:, 0:CS].rearrange("p (h d) -> p h d", h=HPC))
            # QT / KT: (d on partitions, t free)
            for (w_sb, dst) in ((wq_sb, qt_t), (wk_sb, kt_t)):
                for dt_i in range(2):          # two 128-wide d tiles
                    for tch in range(NQB):     # four 512-wide t chunks
                        ps = psum.tile([128, 512], f32, tag="p1", bufs=2,
                                       name="qkps")
                        for k in range(KT_C):
                            nc.tensor.matmul(
                                ps[:],
                                lhsT=w_sb[:, k, dt_i * 128:(dt_i + 1) * 128],
                                rhs=xt_sb[:, k, tch * 512:(tch + 1) * 512],
                                start=(k == 0), stop=(k == KT_C - 1),
                            )
                        for hl in range(2):
                            h = dt_i * 2 + hl
                            d_ap = dst[h][0:64, tch * 512:(tch + 1) * 512]
                            s_ap = ps[hl * 64:(hl + 1) * 64, :]
                            nc.vector.tensor_copy(d_ap, s_ap)

        # ---- phases 2+3 scratch ----
        with tc.tile_pool(name="pt", bufs=3) as pt_pool, \
             tc.tile_pool(name="dn", bufs=4) as dn_pool, \
             tc.tile_pool(name="ot", bufs=2) as ot_pool:
            wo_sb = ot_pool.tile([128, 2, C], f32r)
            nc.gpsimd.dma_start(
                out=wo_sb[:],
                in_=woT.rearrange("(k p) c -> p k c", p=128).bitcast(f32r))

            # ---- phase 2: attention (qb outer) + phase 3 interleaved ----
            for qb in range(NQB):
                for h in range(HPC):
                    n_st = 4 * qb + 4
                    st0 = _st_start(h, qb)
                    av = psum.tile([65, 512], f32, tag="av", bufs=2, name="av")
                    stg0 = st0 - (st0 % 2)
                    for stg in range(stg0, n_st, 2):   # pairs of s-tiles
                        sp = psum.tile([128, 2, 512], f32, tag="st", bufs=2,
                                       name="sp")
                        diag = stg >= 4 * qb
                        offs = []
                        for j in (0, 1):
                            st = stg + j
                            if st < st0:
                                offs.append(None)
                                continue
                            off = _DIAG_OFF[st - 4 * qb] if diag else 0
                            offs.append(off)
                            nc.tensor.matmul(
                                sp[:, j, off:512],
                                lhsT=kt_t[h][:, st * 128:(st + 1) * 128],
                                rhs=qt_t[h][:, qb * 512 + off:(qb + 1) * 512],
                                start=True, stop=True,
                            )
                        pt = pt_pool.tile([128, 2, 512], f32r, tag="pt",
                                          name="ptt")
                        if not diag and offs[0] is not None:
                            nc.scalar.activation(pt[:], sp[:], EXP, scale=SCALE)
                        for j in (0, 1):
                            st = stg + j
                            if offs[j] is None:
                                continue
                            off = offs[j]
                            if diag:  # exp on the valid window, then causal mask
                                r = st - 4 * qb
                                nc.scalar.activation(pt[:, j, off:512],
                                                     sp[:, j, off:512],
                                                     EXP, scale=SCALE)
                                nc.gpsimd.affine_select(
                                    pt[:, j, off:512], pt[:, j, off:512],
                                    pattern=[[1, 512 - off]],
                                    compare_op=mybir.AluOpType.is_ge,
                                    fill=0.0, base=off - 128 * r,
                                    channel_multiplier=-1,
                                )
                            elif not diag and offs[0] is None:
                                nc.scalar.activation(pt[:, j, :], sp[:, j, :],
                                                     EXP, scale=SCALE)
                            nc.tensor.matmul(
                                av[:, off:512],
                                lhsT=v_t[:, st, h, :],
                                rhs=pt[:, j, off:512],
                                start=(st == st0), stop=(st == n_st - 1),
                            )
                    den = dn_pool.tile([1, 512], f32r, tag="den", name="den")
                    nc.vector.reciprocal(den[:], av[64:65, :])
                    bc = psum.tile([64, 512], f32, tag="av", bufs=2, name="bc")
                    nc.tensor.matmul(
                        bc[:], lhsT=ones_sb[:], rhs=den[:], start=True, stop=True)
                    bcs = dn_pool.tile([64, 512], f32, tag="bcs", name="bcs")
                    nc.vector.tensor_copy(bcs[:], bc[:])
                    nc.vector.tensor_mul(
                        attn_t[h // 2][(h % 2) * 64:(h % 2) * 64 + 64,
                                       qb * 512:(qb + 1) * 512],
                        av[0:64, :], bcs[:],
                    )

                # ---- phase 3 strip for this qb: columns complete ----
                for qt_i in range(4 * qb, 4 * qb + 4):
                    ob = ot_pool.tile([128, C], f32, tag="ob", name="ob")
                    for chn in range(2):
                        ps = psum.tile([128, 512], f32, tag="p1", bufs=2,
                                       name="ops")
                        for kt_i in range(2):
                            nc.tensor.matmul(
                                ps[:],
                                lhsT=attn_t[kt_i][:,
                                                  qt_i * 128:(qt_i + 1) * 128],
                                rhs=wo_sb[:, kt_i, chn * 512:(chn + 1) * 512],
                                start=(kt_i == 0), stop=(kt_i == 1),
                            )
                        if chn == 0:
                            nc.vector.tensor_copy(
                                ob[:, chn * 512:(chn + 1) * 512], ps[:])
                        else:
                            nc.scalar.copy(
                                ob[:, chn * 512:(chn + 1) * 512], ps[:])
                    nc.sync.dma_start(
                        out=out[qt_i * 128:(qt_i + 1) * 128, :], in_=ob[:])

    nc.finalize()
    return nc


def _host_inputs(x, Wq, Wk, Wv, Wo):
    """Build the 8 per-core input maps."""
    slopes = _slopes(H)
    t_idx = np.arange(T, dtype=np.float64)
    in_maps = []
    for core in range(NCORES):
        b, g = core // 4, core % 4
        heads = [g + 4 * j for j in range(HPC)]
        hs = np.concatenate([np.arange(h * D, (h + 1) * D) for h in heads])
        sl = slopes[heads].astype(np.float64)
        # qt aug rows per head: [-slope*t/scale, 1, 1]
        # kt aug rows per head: [1, hi(slope*s/scale), lo remainder]
        qaug = np.empty((3 * HPC, T), dtype=np.float32)
        kaug = np.empty((3 * HPC, T), dtype=np.float32)
        for hl in range(HPC):
            v = (sl[hl] * t_idx / SCALE).astype(np.float32)
            hi = _trunc22(v)
            qaug[3 * hl] = (-(sl[hl] * t_idx) / SCALE).astype(np.float32)
            qaug[3 * hl + 1] = 1.0
            qaug[3 * hl + 2] = 1.0
            kaug[3 * hl] = 1.0
            kaug[3 * hl + 1] = hi
            kaug[3 * hl + 2] = (v.astype(np.float64) - hi).astype(np.float32)
        in_maps.append({
            "xT": np.ascontiguousarray(x[b].T),
            "wqT": np.ascontiguousarray(Wq[hs, :].T),
            "wkT": np.ascontiguousarray(Wk[hs, :].T),
            "wvT": np.ascontiguousarray(Wv[hs, :].T),
            "woT": np.ascontiguousarray(Wo[:, hs].T),
            "qaug": qaug,
            "kaug": kaug,
        })
    return in_maps


def get_program():
    global _PROGRAM
    if _PROGRAM is None:
        _PROGRAM = _build_program()
    return _PROGRAM


def kernel(x, Wq, Wk, Wv, Wo, _trace=False):
    from concourse.bass_utils import run_bass_kernel_spmd

    x = np.asarray(x, dtype=np.float32)
    nc = get_program()
    in_maps = _host_inputs(x, np.asarray(Wq, np.float32), np.asarray(Wk, np.float32),
                           np.asarray(Wv, np.float32), np.asarray(Wo, np.float32))
    res = run_bass_kernel_spmd(nc, in_maps, list(range(NCORES)), trace=_trace)
    kernel.last_results = res
    outs = [res.results[i]["out"] for i in range(NCORES)]
    full = np.empty((B, T, C), dtype=np.float32)
    for b in range(B):
        full[b] = outs[4 * b] + outs[4 * b + 1] + outs[4 * b + 2] + outs[4 * b + 3]
    return full
